# revision 18
# baseline (speedup 1.0000x reference)
"""Trainium2 Bass kernel for nn_Critic (gnn_message_passing).

Strategy (8 NeuronCores, one SPMD NEFF):
  - Node-shard the per-node MLPs (8 nodes/core) so the big per-node weights
    (67MB total) are read once across the chip instead of replicated.
  - mm1 in [b, o] layout with a host-augmented "mean" column, LN stats via
    ACT square+accum, PE transpose, fused scale/bias/relu on ACT, mm2 in
    [d, b] layout.
  - Q=A+V and V are downcast to bf16 and AllGathered (DRAM collective).
  - Choquet phase is set-sharded (8 sets/core, 16 instances of Q/V work):
    neighbors gathered by indirect DMA, 2-additive-Mobius pair term via
    delta-batched DVE mins, reductions over d as PSUM-accumulated
    ones-weighted matmuls on the Tensor engine.
All per-core-varying structure (gather indices, Mobius coefficients) enters
as input data so the single SPMD program stays uniform across cores.
"""

import os

import numpy as np

DEBUG = bool(os.environ.get("KERNEL_DEBUG"))
import ml_dtypes

import concourse.bass as bass
import concourse.bacc as bacc
import concourse.mybir as mybir
from concourse import tile
from concourse.bass_utils import run_bass_kernel_spmd

B, N, H, D, K, HEADS = 128, 64, 256, 128, 8, 3
NCORE = 8
NLOC = N // NCORE      # nodes per core
SLOC = N // NCORE      # sets per core
NINST = 2 * SLOC       # choquet instances per core (Q sets then V sets)
NSLOT = K + 1          # center + 8 neighbors
NPAIR = (K * (K - 1)) // 2  # 28
F32 = mybir.dt.float32
BF16 = mybir.dt.bfloat16
I32 = mybir.dt.int32

# pairs in delta-major order over neighbor slots 1..8
PAIRS = [(a, a + d) for d in range(1, K) for a in range(1, K - d + 1)]

_compiled = None


def _build():
    nc = bacc.Bacc("TRN2", target_bir_lowering=False, debug=False,
                   num_devices=NCORE)

    # ---- per-core inputs ----
    obsT = nc.dram_tensor("obsT", [NLOC, H, B], F32, kind="ExternalInput")
    actT = nc.dram_tensor("actT", [NLOC, H, B], F32, kind="ExternalInput")
    vW1 = nc.dram_tensor("vW1", [NLOC, H, H + 1], F32, kind="ExternalInput")
    vb1 = nc.dram_tensor("vb1", [NLOC, H + 1], F32, kind="ExternalInput")
    aW1 = nc.dram_tensor("aW1", [NLOC, 2 * H, H + 1], F32, kind="ExternalInput")
    ab1 = nc.dram_tensor("ab1", [NLOC, H + 1], F32, kind="ExternalInput")
    lnV = nc.dram_tensor("lnV", [B, 4], F32, kind="ExternalInput")
    lnA = nc.dram_tensor("lnA", [B, 4], F32, kind="ExternalInput")
    vW2 = nc.dram_tensor("vW2", [NLOC, 2, 128, D], F32, kind="ExternalInput")
    aW2 = nc.dram_tensor("aW2", [NLOC, 2, 128, D], F32, kind="ExternalInput")
    vb2 = nc.dram_tensor("vb2", [NLOC, D], F32, kind="ExternalInput")
    ab2 = nc.dram_tensor("ab2", [NLOC, D], F32, kind="ExternalInput")
    gidx = nc.dram_tensor("gidx", [B, NINST, NSLOT], I32, kind="ExternalInput")
    pw = nc.dram_tensor("pw", [128, NINST, 7, 4], BF16, kind="ExternalInput")
    sw = nc.dram_tensor("sw", [128, NINST, 3, 4], BF16, kind="ExternalInput")
    ident = nc.dram_tensor("ident", [128, 128], F32, kind="ExternalInput")

    chi = nc.dram_tensor("chi", [1, NINST * B], F32, kind="ExternalOutput")
    if DEBUG:
        dbg_qv = nc.dram_tensor("dbg_qv", [2, NLOC, D, B], BF16,
                                kind="ExternalOutput")
        dbg_x = nc.dram_tensor("dbg_x", [128, NSLOT, B], BF16,
                               kind="ExternalOutput")
        dbg_pm = nc.dram_tensor("dbg_pm", [128, NPAIR, B], BF16,
                                kind="ExternalOutput")
        dbg_c4 = nc.dram_tensor("dbg_c4", [4, 4, B], F32,
                                kind="ExternalOutput")

    with tile.TileContext(nc, num_cores=NCORE) as tc:
        with tc.tile_pool(name="const", bufs=1) as cpool, \
             tc.tile_pool(name="dram", bufs=1, space="DRAM") as dram:
            ident_s = cpool.tile([128, 128], F32)
            nc.sync.dma_start(out=ident_s[:], in_=ident[:])
            ones_row = cpool.tile([1, 128], F32)
            nc.vector.memset(ones_row[:], 1.0)
            eps_t = cpool.tile([B, 1], F32)
            nc.vector.memset(eps_t[:], 1e-5)
            lnV_s = cpool.tile([B, 4], F32)
            nc.sync.dma_start(out=lnV_s[:], in_=lnV[:])
            lnA_s = cpool.tile([B, 4], F32)
            nc.sync.dma_start(out=lnA_s[:], in_=lnA[:])
            pw_s = cpool.tile([128, NINST, 7, 4], BF16)
            nc.sync.dma_start(out=pw_s[:], in_=pw[:])
            sw_s = cpool.tile([128, NINST, 3, 4], BF16)
            nc.sync.dma_start(out=sw_s[:], in_=sw[:])
            gidx_s = cpool.tile([B, NINST, NSLOT], I32)
            nc.sync.dma_start(out=gidx_s[:], in_=gidx[:])

            qvloc = dram.tile([2, NLOC, D, B], BF16)

            # ================= Phase 1: per-node MLPs =================
            with tc.tile_pool(name="p1", bufs=2) as p1, \
                 tc.tile_pool(name="p1w", bufs=2) as p1w, \
                 tc.tile_pool(name="ps_h", bufs=2, space="PSUM") as ps_h, \
                 tc.tile_pool(name="ps_t", bufs=2, space="PSUM") as ps_t, \
                 tc.tile_pool(name="ps_o", bufs=2, space="PSUM") as ps_o:

                def mlp(i, xt_tiles, w1_dram, b1_dram, ln_s, w2_dram, b2_dram,
                        nchunk):
                    # mm1: psum_h[b, o(+mean col)]
                    w1 = p1w.tile([128, nchunk, H + 1], F32, tag="w1")
                    nc.sync.dma_start(
                        out=w1[:],
                        in_=w1_dram[i].rearrange("(c p) o -> p c o", p=128))
                    b1 = p1w.tile([1, H + 1], F32, tag="b1")
                    nc.sync.dma_start(out=b1[:], in_=b1_dram[i][None, :])
                    h = ps_h.tile([B, H + 1], F32, tag="h")
                    for c in range(nchunk):
                        nc.tensor.matmul(h[:], xt_tiles[c], w1[:, c, :],
                                         start=(c == 0), stop=False)
                    nc.tensor.matmul(h[:], ones_row[:], b1[:],
                                     start=False, stop=True)
                    # LN stats
                    mu = p1.tile([B, 1], F32, tag="mu")
                    nc.scalar.copy(mu[:], h[:, H:H + 1])
                    scratch = p1.tile([B, H], F32, tag="scratch")
                    sq = p1.tile([B, 1], F32, tag="sq")
                    nc.scalar.activation(scratch[:], h[:, :H],
                                         mybir.ActivationFunctionType.Square,
                                         accum_out=sq[:])
                    musq = p1.tile([B, 1], F32, tag="musq")
                    nc.scalar.activation(musq[:], mu[:],
                                         mybir.ActivationFunctionType.Square)
                    var = p1.tile([B, 1], F32, tag="var")
                    nc.vector.tensor_scalar(var[:], sq[:], 1.0 / H, None,
                                            mybir.AluOpType.mult)
                    nc.vector.tensor_tensor(var[:], var[:], musq[:],
                                            mybir.AluOpType.subtract)
                    sd = p1.tile([B, 1], F32, tag="sd")
                    nc.scalar.activation(sd[:], var[:],
                                         mybir.ActivationFunctionType.Sqrt,
                                         bias=eps_t[:])
                    rs = p1.tile([B, 1], F32, tag="rs")
                    nc.vector.reciprocal(rs[:], sd[:])
                    # apply (h-mu)*rs
                    u = p1.tile([B, H], F32, tag="u")
                    nc.vector.tensor_scalar(u[:], h[:, :H], mu[:], rs[:],
                                            mybir.AluOpType.subtract,
                                            mybir.AluOpType.mult)
                    # transpose u -> uT, relu(g*uT+be) -> hT
                    ut = ps_t.tile([128, 2, 128], F32, tag="ut")
                    for c in range(2):
                        nc.tensor.transpose(ut[:, c, :],
                                            u[:, c * 128:(c + 1) * 128],
                                            ident_s[:])
                    hT = p1.tile([128, 2, 128], F32, tag="hT")
                    for c in range(2):
                        nc.scalar.activation(hT[:, c, :], ut[:, c, :],
                                             mybir.ActivationFunctionType.Relu,
                                             bias=ln_s[:, 2 + c:3 + c],
                                             scale=ln_s[:, c:c + 1])
                    # mm2: out[d, b]
                    w2 = p1w.tile([128, 2, D], F32, tag="w2")
                    nc.sync.dma_start(
                        out=w2[:],
                        in_=w2_dram[i].rearrange("c p d -> p c d"))
                    b2 = p1w.tile([1, D], F32, tag="b2")
                    nc.sync.dma_start(out=b2[:], in_=b2_dram[i][None, :])
                    o = ps_o.tile([D, B], F32, tag="o")
                    for c in range(2):
                        nc.tensor.matmul(o[:], w2[:, c, :], hT[:, c, :],
                                         start=(c == 0), stop=False)
                    nc.tensor.matmul(o[:], b2[:], ones_row[:],
                                     start=False, stop=True)
                    return o

                for i in range(NLOC):
                    xv = p1.tile([128, 2, B], F32, tag="xv")
                    nc.sync.dma_start(
                        out=xv[:],
                        in_=obsT[i].rearrange("(c p) b -> p c b", p=128))
                    xa = p1.tile([128, 2, B], F32, tag="xa")
                    nc.sync.dma_start(
                        out=xa[:],
                        in_=actT[i].rearrange("(c p) b -> p c b", p=128))
                    ov = mlp(i, [xv[:, 0, :], xv[:, 1, :]],
                             vW1, vb1, lnV_s, vW2, vb2, 2)
                    oa = mlp(i, [xv[:, 0, :], xv[:, 1, :],
                                 xa[:, 0, :], xa[:, 1, :]],
                             aW1, ab1, lnA_s, aW2, ab2, 4)
                    vf = p1.tile([D, B], F32, tag="vf")
                    nc.scalar.copy(vf[:], ov[:])
                    qb = p1.tile([D, B], BF16, tag="qb")
                    nc.vector.tensor_tensor(qb[:], oa[:], vf[:],
                                            mybir.AluOpType.add)
                    vb = p1.tile([D, B], BF16, tag="vb")
                    nc.vector.tensor_scalar(vb[:], vf[:], 1.0, None,
                                            mybir.AluOpType.mult)
                    nc.sync.dma_start(out=qvloc[0, i], in_=qb[:])
                    nc.sync.dma_start(out=qvloc[1, i], in_=vb[:])

            # ================= Phase 2: AllGather =================
            if DEBUG:
                nc.sync.dma_start(out=dbg_qv[:], in_=qvloc[:])
            qvall = dram.tile([NCORE, 2, NLOC, D, B], BF16)
            nc.gpsimd.collective_compute(
                "AllGather", mybir.AluOpType.bypass,
                replica_groups=[list(range(NCORE))],
                ins=[qvloc.opt()], outs=[qvall.opt()],
            )
            qv_rows = qvall.rearrange("c t n p b -> (c t n p) b")

            # ================= Phase 3: Choquet =================
            with tc.tile_pool(name="p3", bufs=3) as p3, \
                 tc.tile_pool(name="ps_p", bufs=4, space="PSUM") as ps_p, \
                 tc.tile_pool(name="ps_r", bufs=1, space="PSUM") as ps_r, \
                 tc.tile_pool(name="p3c", bufs=1) as p3c:
                ones4 = p3c.tile([4, 1], F32)
                nc.vector.memset(ones4[:], 1.0)
                chirow = ps_r.tile([1, NINST * B], F32)
                chi4 = [p3c.tile([4, NINST // 2, 4, B], F32,
                                 name="chi4a", tag="chi4a"),
                        p3c.tile([4, NINST // 2, 4, B], F32,
                                 name="chi4b", tag="chi4b")]
                for i in range(NINST):
                    X = p3.tile([128, NSLOT, B], BF16, tag="X")
                    # HW indirect DMA contract: one index per partition
                    for k in range(NSLOT):
                        nc.gpsimd.indirect_dma_start(
                            out=X[:, k, :], out_offset=None,
                            in_=qv_rows,
                            in_offset=bass.IndirectOffsetOnAxis(
                                ap=gidx_s[:, i, k:k + 1], axis=0),
                        )
                    PM = p3.tile([128, NPAIR, B], BF16, tag="PM")
                    off = 0
                    for d in range(1, K):
                        n = K - d
                        nc.vector.tensor_tensor(
                            PM[:, off:off + n, :],
                            X[:, 1:1 + n, :], X[:, 1 + d:1 + d + n, :],
                            mybir.AluOpType.min)
                        off += n
                    if DEBUG and i == 0:
                        nc.sync.dma_start(out=dbg_x[:], in_=X[:])
                        nc.sync.dma_start(out=dbg_pm[:], in_=PM[:])
                    P = ps_p.tile([4, 4 * B], F32, tag="P")
                    for j in range(7):
                        nc.tensor.matmul(
                            P[:], pw_s[:, i, j, :],
                            PM[:, 4 * j:4 * j + 4, :].rearrange(
                                "p a b -> p (a b)"),
                            start=(j == 0), stop=False)
                    # slot groups 0-3, 4-7, 5-8 (last has zero coeffs on 5-7)
                    for j, s0 in enumerate((0, 4, 5)):
                        nc.tensor.matmul(
                            P[:], sw_s[:, i, j, :],
                            X[:, s0:s0 + 4, :].rearrange("p a b -> p (a b)"),
                            start=False, stop=(j == 2))
                    # copy psum -> sbuf (two independent tiles: ACT and DVE)
                    dst = chi4[i % 2][:, i // 2, :, :].rearrange(
                        "p a b -> p (a b)")
                    if i % 2 == 0:
                        nc.scalar.copy(dst, P[:])
                    else:
                        nc.vector.tensor_copy(dst, P[:])
                if DEBUG:
                    nc.sync.dma_start(
                        out=dbg_c4[:],
                        in_=chi4[0][:, 0, :, :])
                # P[c, block, b] is valid only on the diagonal block==c
                # (stationary col c pairs with rhs block c); fold with
                # one-hot columns to extract P[c, c, b] and sum over c.
                for i in range(NINST):
                    for c in range(4):
                        nc.tensor.matmul(
                            chirow[:, i * B:(i + 1) * B],
                            ident_s[:4, c:c + 1],
                            chi4[i % 2][:, i // 2, c, :],
                            start=(c == 0), stop=(c == 3))
                chirow_s = p3c.tile([1, NINST * B], F32)
                nc.scalar.copy(chirow_s[:], chirow[:])
                nc.sync.dma_start(out=chi[:], in_=chirow_s[:])

    nc.compile()
    return nc


def _prepare_inputs(observation, action, local_edges, V_W1, V_b1, V_g1,
                    V_beta1, V_W2, V_b2, A_W1, A_b1, A_g1, A_beta1, A_W2,
                    A_b2, chi_m1, chi_m2):
    centers = np.asarray(local_edges[:, 0, 0]).astype(np.int64)
    neigh = np.asarray(local_edges[:, 0, 1:]).astype(np.int64)
    m1s = chi_m1.sum(1) / (HEADS * D)              # [S, K]
    tri = np.triu(np.ones((K, K), np.float32), k=1)
    m2s = (chi_m2.sum(1) * tri) / (HEADS * D)      # [S, K, K]

    in_maps = []
    for c in range(NCORE):
        nodes = slice(c * NLOC, (c + 1) * NLOC)
        obsn = observation[:, nodes, :]            # [B, 8, H]
        actn = action[:, nodes, :]
        m = {}
        m["obsT"] = np.ascontiguousarray(obsn.transpose(1, 2, 0))
        m["actT"] = np.ascontiguousarray(actn.transpose(1, 2, 0))
        w1 = V_W1[nodes]                           # [8, H, H]
        m["vW1"] = np.ascontiguousarray(
            np.concatenate([w1, w1.mean(2, keepdims=True)], 2))
        m["vb1"] = np.ascontiguousarray(
            np.concatenate([V_b1[nodes],
                            V_b1[nodes].mean(1, keepdims=True)], 1))
        w1a = A_W1[nodes]                          # [8, 2H, H]
        m["aW1"] = np.ascontiguousarray(
            np.concatenate([w1a, w1a.mean(2, keepdims=True)], 2))
        m["ab1"] = np.ascontiguousarray(
            np.concatenate([A_b1[nodes],
                            A_b1[nodes].mean(1, keepdims=True)], 1))
        lnv = np.zeros((B, 4), np.float32)
        lnv[:, 0] = V_g1[:128]; lnv[:, 1] = V_g1[128:]
        lnv[:, 2] = V_beta1[:128]; lnv[:, 3] = V_beta1[128:]
        m["lnV"] = lnv
        lna = np.zeros((B, 4), np.float32)
        lna[:, 0] = A_g1[:128]; lna[:, 1] = A_g1[128:]
        lna[:, 2] = A_beta1[:128]; lna[:, 3] = A_beta1[128:]
        m["lnA"] = lna
        m["vW2"] = np.ascontiguousarray(
            V_W2[nodes].reshape(NLOC, 2, 128, D))
        m["aW2"] = np.ascontiguousarray(
            A_W2[nodes].reshape(NLOC, 2, 128, D))
        m["vb2"] = np.ascontiguousarray(V_b2[nodes])
        m["ab2"] = np.ascontiguousarray(A_b2[nodes])

        gi = np.zeros((B, NINST, NSLOT), np.int32)
        pwn = np.zeros((NINST, 7, 4), np.float32)
        swn = np.zeros((NINST, 3, 4), np.float32)
        for i in range(NINST):
            t = 0 if i < SLOC else 1               # 0=Q, 1=V
            s = c * SLOC + (i % SLOC)
            slots = [centers[s]] + list(neigh[s])
            for k in range(NSLOT):
                node = slots[k]
                row = ((node // NLOC) * 2 + t) * NLOC + (node % NLOC)
                gi[:, i, k] = row * D + np.arange(B)
            for p, (a, b_) in enumerate(PAIRS):
                pwn[i, p // 4, p % 4] = m2s[s, a - 1, b_ - 1]
            # device slot groups: j=0 slots 0-3, j=1 slots 4-7, j=2 slots 5-8
            swn[i, 0, 0] = 1.0 / D                 # center
            for k in range(1, 8):
                swn[i, k // 4, k % 4] = m1s[s, k - 1]
            swn[i, 2, 3] = m1s[s, 7]               # slot 8
        m["gidx"] = gi
        m["pw"] = np.broadcast_to(
            pwn.astype(ml_dtypes.bfloat16)[None], (128, NINST, 7, 4)).copy()
        m["sw"] = np.broadcast_to(
            swn.astype(ml_dtypes.bfloat16)[None], (128, NINST, 3, 4)).copy()
        m["ident"] = np.eye(128, dtype=np.float32)
        in_maps.append(m)
    return in_maps


def kernel(**inputs):
    global _compiled
    if _compiled is None:
        _compiled = _build()
    nc = _compiled
    inputs = {k: np.asarray(v) for k, v in inputs.items()}
    in_maps = _prepare_inputs(**inputs)
    res = run_bass_kernel_spmd(nc, in_maps, list(range(NCORE)))
    global _last_results
    _last_results = res
    chi_q = np.zeros((B, N), np.float32)
    chi_v = np.zeros((B, N), np.float32)
    for c in range(NCORE):
        out = res.results[c]["chi"].reshape(NINST, B)
        for sl in range(SLOC):
            chi_q[:, c * SLOC + sl] = out[sl]
            chi_v[:, c * SLOC + sl] = out[SLOC + sl]
    return chi_q, chi_v


# revision 20
# speedup vs baseline: 1.2265x; 1.2265x over previous
"""Trainium2 Bass kernel for nn_Critic (gnn_message_passing).

Strategy (8 NeuronCores, one SPMD NEFF):
  - Node-shard the per-node MLPs (8 nodes/core) so the big per-node weights
    (67MB total) are read once across the chip instead of replicated.
  - mm1 in [b, o] layout with a host-augmented "mean" column, LN stats via
    ACT square+accum, rsqrt as exp(-0.5*ln(var+eps)) (keeps every ACT
    function in one LUT table-set), PE transpose, fused scale/bias/relu on
    ACT, mm2 in [d, b] layout.
  - Q=A+V and V are packed [node, d, {q,v}, b] in bf16 and AllGathered.
  - Choquet phase is set-sharded (8 sets/core; Q and V ride together):
    neighbor blocks fetched as contiguous 64KB register-offset HWDGE DMAs
    (offsets come from an int32 input -> values_load), pair terms via
    delta-batched DVE mins, and all d-reductions as PSUM-accumulated
    weighted matmuls on the Tensor engine (diagonal-block extraction).
All per-core-varying structure (gather offsets, Mobius coefficients) enters
as input data so the single SPMD program stays uniform across cores.
"""

import os

import numpy as np
import ml_dtypes

import concourse.bass as bass
import concourse.bacc as bacc
import concourse.mybir as mybir
from concourse import tile
from concourse.bass_utils import run_bass_kernel_spmd

DEBUG = bool(os.environ.get("KERNEL_DEBUG"))

B, N, H, D, K, HEADS = 128, 64, 256, 128, 8, 3
NCORE = 8
NLOC = N // NCORE      # nodes per core
SLOC = N // NCORE      # sets per core
NINST = 2 * SLOC       # choquet instances per core (Q sets then V sets)
NSLOT = K + 1          # center + 8 neighbors
NPAIR = (K * (K - 1)) // 2  # 28
F32 = mybir.dt.float32
BF16 = mybir.dt.bfloat16
I32 = mybir.dt.int32

# pairs in delta-major order over neighbor slots 1..8
PAIRS = [(a, a + d) for d in range(1, K) for a in range(1, K - d + 1)]

_compiled = None


def _build():
    nc = bacc.Bacc("TRN2", target_bir_lowering=False, debug=False,
                   num_devices=NCORE)

    # ---- per-core inputs ----
    obsT = nc.dram_tensor("obsT", [NLOC, H, B], F32, kind="ExternalInput")
    actT = nc.dram_tensor("actT", [NLOC, H, B], F32, kind="ExternalInput")
    vW1 = nc.dram_tensor("vW1", [NLOC, H, H + 1], F32, kind="ExternalInput")
    vb1 = nc.dram_tensor("vb1", [NLOC, H + 1], F32, kind="ExternalInput")
    aW1 = nc.dram_tensor("aW1", [NLOC, 2 * H, H + 1], F32, kind="ExternalInput")
    ab1 = nc.dram_tensor("ab1", [NLOC, H + 1], F32, kind="ExternalInput")
    lnV = nc.dram_tensor("lnV", [B, 4], F32, kind="ExternalInput")
    lnA = nc.dram_tensor("lnA", [B, 4], F32, kind="ExternalInput")
    vW2 = nc.dram_tensor("vW2", [NLOC, 2, 128, D], F32, kind="ExternalInput")
    aW2 = nc.dram_tensor("aW2", [NLOC, 2, 128, D], F32, kind="ExternalInput")
    vb2 = nc.dram_tensor("vb2", [NLOC, D], F32, kind="ExternalInput")
    ab2 = nc.dram_tensor("ab2", [NLOC, D], F32, kind="ExternalInput")
    # row-base offsets (node*128) for each (set, slot)
    gbase = nc.dram_tensor("gbase", [1, SLOC * NSLOT], I32,
                           kind="ExternalInput")
    pw = nc.dram_tensor("pw", [128, SLOC, 7, 4], BF16, kind="ExternalInput")
    sw = nc.dram_tensor("sw", [128, SLOC, 3, 4], BF16, kind="ExternalInput")
    ident = nc.dram_tensor("ident", [128, 128], F32, kind="ExternalInput")

    chi = nc.dram_tensor("chi", [1, NINST * B], F32, kind="ExternalOutput")
    if DEBUG:
        dbg_qv = nc.dram_tensor("dbg_qv", [NLOC, D, 2, B], BF16,
                                kind="ExternalOutput")
        dbg_x = nc.dram_tensor("dbg_x", [128, NSLOT, 2, B], BF16,
                               kind="ExternalOutput")
        dbg_pm = nc.dram_tensor("dbg_pm", [128, NPAIR, 2, B], BF16,
                                kind="ExternalOutput")
        dbg_c4 = nc.dram_tensor("dbg_c4", [4, 4, B], F32,
                                kind="ExternalOutput")

    with tile.TileContext(nc, num_cores=NCORE) as tc:
        with tc.tile_pool(name="const", bufs=1) as cpool, \
             tc.tile_pool(name="dram", bufs=1, space="DRAM") as dram:
            ident_s = cpool.tile([128, 128], F32)
            nc.sync.dma_start(out=ident_s[:], in_=ident[:])
            ones_row = cpool.tile([1, 128], F32)
            nc.vector.memset(ones_row[:], 1.0)
            eps_t = cpool.tile([B, 1], F32)
            nc.vector.memset(eps_t[:], 1e-5)
            lnV_s = cpool.tile([B, 4], F32)
            nc.sync.dma_start(out=lnV_s[:], in_=lnV[:])
            lnA_s = cpool.tile([B, 4], F32)
            nc.sync.dma_start(out=lnA_s[:], in_=lnA[:])
            pw_s = cpool.tile([128, SLOC, 7, 4], BF16)
            nc.sync.dma_start(out=pw_s[:], in_=pw[:])
            sw_s = cpool.tile([128, SLOC, 3, 4], BF16)
            nc.sync.dma_start(out=sw_s[:], in_=sw[:])
            gbase_s = cpool.tile([1, SLOC * NSLOT], I32)
            nc.sync.dma_start(out=gbase_s[:], in_=gbase[:])

            qvloc = dram.tile([NLOC, D, 2, B], BF16)

            # ================= Phase 1: per-node MLPs =================
            with tc.tile_pool(name="p1", bufs=3) as p1, \
                 tc.tile_pool(name="p1w", bufs=3) as p1w, \
                 tc.tile_pool(name="ps_h", bufs=3, space="PSUM") as ps_h, \
                 tc.tile_pool(name="ps_t", bufs=2, space="PSUM") as ps_t, \
                 tc.tile_pool(name="ps_o", bufs=2, space="PSUM") as ps_o:

                def mlp(i, xt_tiles, w1_dram, b1_dram, ln_s, w2_dram, b2_dram,
                        nchunk):
                    # mm1: psum_h[b, o | mean col]
                    w1 = p1w.tile([128, nchunk, H + 1], F32, tag="w1")
                    nc.sync.dma_start(
                        out=w1[:],
                        in_=w1_dram[i].rearrange("(c p) o -> p c o", p=128))
                    b1 = p1w.tile([1, H + 1], F32, tag="b1")
                    nc.sync.dma_start(out=b1[:], in_=b1_dram[i][None, :])
                    h = ps_h.tile([B, H + 1], F32, tag="h")
                    for c in range(nchunk):
                        nc.tensor.matmul(h[:], xt_tiles[c], w1[:, c, :],
                                         start=(c == 0), stop=False)
                    nc.tensor.matmul(h[:], ones_row[:], b1[:],
                                     start=False, stop=True)
                    # LN stats: mean from the augmented col, Sum(h^2) via ACT
                    mu = p1.tile([B, 1], F32, tag="mu")
                    nc.vector.tensor_scalar(mu[:], h[:, H:H + 1], 1.0, None,
                                            mybir.AluOpType.mult)
                    scratch = p1.tile([B, H], F32, tag="scratch")
                    sq = p1.tile([B, 1], F32, tag="sq")
                    nc.scalar.activation(scratch[:], h[:, :H],
                                         mybir.ActivationFunctionType.Square,
                                         accum_out=sq[:])
                    var = p1.tile([B, 1], F32, tag="var")
                    musq = p1.tile([B, 1], F32, tag="musq")
                    nc.vector.tensor_tensor(musq[:], mu[:], mu[:],
                                            mybir.AluOpType.mult)
                    nc.vector.tensor_scalar(var[:], sq[:], 1.0 / H, None,
                                            mybir.AluOpType.mult)
                    nc.vector.tensor_tensor(var[:], var[:], musq[:],
                                            mybir.AluOpType.subtract)
                    # rs = 1/sqrt(var+eps) = exp(-0.5*ln(var+eps))
                    lv = p1.tile([B, 1], F32, tag="lv")
                    nc.scalar.activation(lv[:], var[:],
                                         mybir.ActivationFunctionType.Ln,
                                         bias=eps_t[:])
                    rs = p1.tile([B, 1], F32, tag="rs")
                    nc.scalar.activation(rs[:], lv[:],
                                         mybir.ActivationFunctionType.Exp,
                                         scale=-0.5)
                    # apply (h-mu)*rs
                    u = p1.tile([B, H], F32, tag="u")
                    nc.vector.tensor_scalar(u[:], h[:, :H], mu[:], rs[:],
                                            mybir.AluOpType.subtract,
                                            mybir.AluOpType.mult)
                    # transpose u -> uT, relu(g*uT+be) -> hT
                    ut = ps_t.tile([128, 2, 128], F32, tag="ut")
                    for c in range(2):
                        nc.tensor.transpose(ut[:, c, :],
                                            u[:, c * 128:(c + 1) * 128],
                                            ident_s[:])
                    hT = p1.tile([128, 2, 128], F32, tag="hT")
                    for c in range(2):
                        nc.scalar.activation(hT[:, c, :], ut[:, c, :],
                                             mybir.ActivationFunctionType.Relu,
                                             bias=ln_s[:, 2 + c:3 + c],
                                             scale=ln_s[:, c:c + 1])
                    # mm2: out[d, b]
                    w2 = p1w.tile([128, 2, D], F32, tag="w2")
                    nc.sync.dma_start(
                        out=w2[:],
                        in_=w2_dram[i].rearrange("c p d -> p c d"))
                    b2 = p1w.tile([1, D], F32, tag="b2")
                    nc.sync.dma_start(out=b2[:], in_=b2_dram[i][None, :])
                    o = ps_o.tile([D, B], F32, tag="o")
                    for c in range(2):
                        nc.tensor.matmul(o[:], w2[:, c, :], hT[:, c, :],
                                         start=(c == 0), stop=False)
                    nc.tensor.matmul(o[:], b2[:], ones_row[:],
                                     start=False, stop=True)
                    return o

                for i in range(NLOC):
                    xv = p1.tile([128, 2, B], F32, tag="xv")
                    nc.sync.dma_start(
                        out=xv[:],
                        in_=obsT[i].rearrange("(c p) b -> p c b", p=128))
                    xa = p1.tile([128, 2, B], F32, tag="xa")
                    nc.sync.dma_start(
                        out=xa[:],
                        in_=actT[i].rearrange("(c p) b -> p c b", p=128))
                    ov = mlp(i, [xv[:, 0, :], xv[:, 1, :]],
                             vW1, vb1, lnV_s, vW2, vb2, 2)
                    oa = mlp(i, [xv[:, 0, :], xv[:, 1, :],
                                 xa[:, 0, :], xa[:, 1, :]],
                             aW1, ab1, lnA_s, aW2, ab2, 4)
                    vf = p1.tile([D, B], F32, tag="vf")
                    nc.scalar.copy(vf[:], ov[:])
                    qb = p1.tile([D, B], BF16, tag="qb")
                    nc.vector.tensor_tensor(qb[:], oa[:], vf[:],
                                            mybir.AluOpType.add)
                    vb = p1.tile([D, B], BF16, tag="vb")
                    nc.vector.tensor_scalar(vb[:], vf[:], 1.0, None,
                                            mybir.AluOpType.mult)
                    nc.sync.dma_start(out=qvloc[i, :, 0, :], in_=qb[:])
                    nc.sync.dma_start(out=qvloc[i, :, 1, :], in_=vb[:])

            # ================= Phase 2: AllGather =================
            if DEBUG:
                nc.sync.dma_start(out=dbg_qv[:], in_=qvloc[:])
            qvall = dram.tile([NCORE, NLOC, D, 2, B], BF16)
            nc.gpsimd.collective_compute(
                "AllGather", mybir.AluOpType.bypass,
                replica_groups=[list(range(NCORE))],
                ins=[qvloc.opt()], outs=[qvall.opt()],
            )
            # flat rows: row (node, d) = node*128 + d, each row [2, B] = 256
            qv_flat = qvall.rearrange("c n p t b -> (c n p) (t b)")

            # ================= Phase 3: Choquet =================
            with tc.tile_pool(name="p3", bufs=4) as p3, \
                 tc.tile_pool(name="p3pm", bufs=2) as p3pm, \
                 tc.tile_pool(name="ps_p", bufs=4, space="PSUM") as ps_p, \
                 tc.tile_pool(name="ps_r", bufs=1, space="PSUM") as ps_r, \
                 tc.tile_pool(name="p3c", bufs=1) as p3c:
                chirow = ps_r.tile([1, NINST * B], F32)
                chi4 = [p3c.tile([4, SLOC, 4, B], F32,
                                 name="chi4q", tag="chi4q"),
                        p3c.tile([4, SLOC, 4, B], F32,
                                 name="chi4v", tag="chi4v")]
                for s in range(SLOC):
                    X = p3.tile([128, NSLOT, 2, B], BF16, tag="X")
                    for k in range(NSLOT):
                        base = nc.values_load(
                            gbase_s[0:1, s * NSLOT + k:s * NSLOT + k + 1],
                            engines=[mybir.EngineType.SP],
                            skip_runtime_bounds_check=True)
                        nc.sync.dma_start(
                            out=X[:, k, :, :],
                            in_=qv_flat[bass.ds(base, 128), :])
                    PM = p3pm.tile([128, NPAIR, 2, B], BF16, tag="PM")
                    off = 0
                    for dd in range(1, K):
                        n = K - dd
                        nc.vector.tensor_tensor(
                            PM[:, off:off + n, :, :],
                            X[:, 1:1 + n, :, :], X[:, 1 + dd:1 + dd + n, :, :],
                            mybir.AluOpType.min)
                        off += n
                    if DEBUG and s == 0:
                        nc.sync.dma_start(out=dbg_x[:], in_=X[:])
                        nc.sync.dma_start(out=dbg_pm[:], in_=PM[:])
                    for t in range(2):
                        P = ps_p.tile([4, 4 * B], F32, tag="P")
                        for j in range(7):
                            nc.tensor.matmul(
                                P[:], pw_s[:, s, j, :], PM[:, 4 * j:4 * j + 4, t, :],
                                start=(j == 0), stop=False)
                        for j, s0 in enumerate((0, 4, 5)):
                            nc.tensor.matmul(
                                P[:], sw_s[:, s, j, :], X[:, s0:s0 + 4, t, :],
                                start=False, stop=(j == 2))
                        # copy psum -> sbuf (Q tile via ACT, V tile via DVE)
                        dst = chi4[t][:, s, :, :].rearrange("p a b -> p (a b)")
                        if t == 0:
                            nc.scalar.copy(dst, P[:])
                        else:
                            nc.vector.tensor_copy(dst, P[:])
                if DEBUG:
                    nc.sync.dma_start(out=dbg_c4[:], in_=chi4[0][:, 0, :, :])
                # fold: extract diagonal blocks P[c, c, b]; batch 4 sets/MM
                for t in range(2):
                    for g in range(SLOC // 4):
                        for c in range(4):
                            nc.tensor.matmul(
                                chirow[:, (t * SLOC + g * 4) * B:
                                       (t * SLOC + g * 4 + 4) * B],
                                ident_s[:4, c:c + 1],
                                chi4[t][:, 4 * g:4 * g + 4, c, :],
                                start=(c == 0), stop=(c == 3))
                chirow_s = p3c.tile([1, NINST * B], F32)
                nc.scalar.copy(chirow_s[:], chirow[:])
                nc.sync.dma_start(out=chi[:], in_=chirow_s[:])

    nc.compile()
    return nc


def _prepare_inputs(observation, action, local_edges, V_W1, V_b1, V_g1,
                    V_beta1, V_W2, V_b2, A_W1, A_b1, A_g1, A_beta1, A_W2,
                    A_b2, chi_m1, chi_m2):
    centers = np.asarray(local_edges[:, 0, 0]).astype(np.int64)
    neigh = np.asarray(local_edges[:, 0, 1:]).astype(np.int64)
    m1s = chi_m1.sum(1) / (HEADS * D)              # [S, K]
    tri = np.triu(np.ones((K, K), np.float32), k=1)
    m2s = (chi_m2.sum(1) * tri) / (HEADS * D)      # [S, K, K]

    in_maps = []
    for c in range(NCORE):
        nodes = slice(c * NLOC, (c + 1) * NLOC)
        obsn = observation[:, nodes, :]            # [B, 8, H]
        actn = action[:, nodes, :]
        m = {}
        m["obsT"] = np.ascontiguousarray(obsn.transpose(1, 2, 0))
        m["actT"] = np.ascontiguousarray(actn.transpose(1, 2, 0))
        w1 = V_W1[nodes]                           # [8, H, H]
        m["vW1"] = np.ascontiguousarray(
            np.concatenate([w1, w1.mean(2, keepdims=True)], 2))
        m["vb1"] = np.ascontiguousarray(
            np.concatenate([V_b1[nodes],
                            V_b1[nodes].mean(1, keepdims=True)], 1))
        w1a = A_W1[nodes]                          # [8, 2H, H]
        m["aW1"] = np.ascontiguousarray(
            np.concatenate([w1a, w1a.mean(2, keepdims=True)], 2))
        m["ab1"] = np.ascontiguousarray(
            np.concatenate([A_b1[nodes],
                            A_b1[nodes].mean(1, keepdims=True)], 1))
        lnv = np.zeros((B, 4), np.float32)
        lnv[:, 0] = V_g1[:128]; lnv[:, 1] = V_g1[128:]
        lnv[:, 2] = V_beta1[:128]; lnv[:, 3] = V_beta1[128:]
        m["lnV"] = lnv
        lna = np.zeros((B, 4), np.float32)
        lna[:, 0] = A_g1[:128]; lna[:, 1] = A_g1[128:]
        lna[:, 2] = A_beta1[:128]; lna[:, 3] = A_beta1[128:]
        m["lnA"] = lna
        m["vW2"] = np.ascontiguousarray(
            V_W2[nodes].reshape(NLOC, 2, 128, D))
        m["aW2"] = np.ascontiguousarray(
            A_W2[nodes].reshape(NLOC, 2, 128, D))
        m["vb2"] = np.ascontiguousarray(V_b2[nodes])
        m["ab2"] = np.ascontiguousarray(A_b2[nodes])

        gb = np.zeros((1, SLOC * NSLOT), np.int32)
        pwn = np.zeros((SLOC, 7, 4), np.float32)
        swn = np.zeros((SLOC, 3, 4), np.float32)
        for sl in range(SLOC):
            s = c * SLOC + sl
            slots = [int(centers[s])] + [int(x) for x in neigh[s]]
            for k in range(NSLOT):
                gb[0, sl * NSLOT + k] = slots[k] * D
            for p, (a, b_) in enumerate(PAIRS):
                pwn[sl, p // 4, p % 4] = m2s[s, a - 1, b_ - 1]
            # device slot groups: j=0 slots 0-3, j=1 slots 4-7, j=2 slots 5-8
            swn[sl, 0, 0] = 1.0 / D                # center
            for k in range(1, 8):
                swn[sl, k // 4, k % 4] = m1s[s, k - 1]
            swn[sl, 2, 3] = m1s[s, 7]              # slot 8
        m["gbase"] = gb
        m["pw"] = np.broadcast_to(
            pwn.astype(ml_dtypes.bfloat16)[None], (128, SLOC, 7, 4)).copy()
        m["sw"] = np.broadcast_to(
            swn.astype(ml_dtypes.bfloat16)[None], (128, SLOC, 3, 4)).copy()
        m["ident"] = np.eye(128, dtype=np.float32)
        in_maps.append(m)
    return in_maps


def kernel(**inputs):
    global _compiled
    if _compiled is None:
        _compiled = _build()
    nc = _compiled
    inputs = {k: np.asarray(v) for k, v in inputs.items()}
    in_maps = _prepare_inputs(**inputs)
    res = run_bass_kernel_spmd(nc, in_maps, list(range(NCORE)))
    global _last_results
    _last_results = res
    chi_q = np.zeros((B, N), np.float32)
    chi_v = np.zeros((B, N), np.float32)
    for c in range(NCORE):
        out = res.results[c]["chi"].reshape(NINST, B)
        for sl in range(SLOC):
            chi_q[:, c * SLOC + sl] = out[sl]
            chi_v[:, c * SLOC + sl] = out[SLOC + sl]
    return chi_q, chi_v


# revision 21
# speedup vs baseline: 1.4915x; 1.2160x over previous
"""Trainium2 Bass kernel for nn_Critic (gnn_message_passing).

Strategy (8 NeuronCores, one SPMD NEFF):
  - Node-shard the per-node MLPs (8 nodes/core) so the big per-node weights
    (67MB total) are read once across the chip instead of replicated.
  - mm1 in [b, o] layout, LN stats via DVE bn_stats/bn_aggr, rsqrt as
    exp(-0.5*ln(var+eps)) (keeps every ACT function in one LUT table-set),
    PE transpose, fused scale/bias/relu on ACT, mm2 in [d, b] layout.
  - Q=A+V and V are packed [node, d, {q,v}, b] in bf16 and AllGathered in
    two halves so the first collective overlaps the second half of the MLP
    compute.
  - Choquet phase is set-sharded (8 sets/core; Q and V ride together):
    neighbor blocks fetched as contiguous 64KB register-offset HWDGE DMAs
    (offsets from an int32 input -> batched reg loads, alternating the
    sync/scalar DMA queues), pair terms via delta-batched DVE mins over
    two-set groups, and all d-reductions as PSUM-accumulated weighted
    matmuls on the Tensor engine (diagonal-block extraction).
All per-core-varying structure (gather offsets, Mobius coefficients) enters
as input data so the single SPMD program stays uniform across cores.
"""

import os

import numpy as np
import ml_dtypes

import concourse.bass as bass
import concourse.bacc as bacc
import concourse.mybir as mybir
from concourse import tile
from concourse.bass_utils import run_bass_kernel_spmd

DEBUG = bool(os.environ.get("KERNEL_DEBUG"))

B, N, H, D, K, HEADS = 128, 64, 256, 128, 8, 3
NCORE = 8
NLOC = N // NCORE      # nodes per core
SLOC = N // NCORE      # sets per core
NINST = 2 * SLOC       # choquet instances per core (Q sets then V sets)
NSLOT = K + 1          # center + 8 neighbors
NPAIR = (K * (K - 1)) // 2  # 28
NH = NLOC // 2         # nodes per collective half
F32 = mybir.dt.float32
BF16 = mybir.dt.bfloat16
I32 = mybir.dt.int32

# pairs in delta-major order over neighbor slots 1..8
PAIRS = [(a, a + d) for d in range(1, K) for a in range(1, K - d + 1)]

_compiled = None


def _build():
    nc = bacc.Bacc("TRN2", target_bir_lowering=False, debug=False,
                   num_devices=NCORE)

    # ---- per-core inputs ----
    obsT = nc.dram_tensor("obsT", [NLOC, H, B], F32, kind="ExternalInput")
    actT = nc.dram_tensor("actT", [NLOC, H, B], F32, kind="ExternalInput")
    vW1 = nc.dram_tensor("vW1", [NLOC, H, H], F32, kind="ExternalInput")
    vb1 = nc.dram_tensor("vb1", [NLOC, H], F32, kind="ExternalInput")
    aW1 = nc.dram_tensor("aW1", [NLOC, 2 * H, H], F32, kind="ExternalInput")
    ab1 = nc.dram_tensor("ab1", [NLOC, H], F32, kind="ExternalInput")
    lnV = nc.dram_tensor("lnV", [B, 4], F32, kind="ExternalInput")
    lnA = nc.dram_tensor("lnA", [B, 4], F32, kind="ExternalInput")
    vW2 = nc.dram_tensor("vW2", [NLOC, 2, 128, D], F32, kind="ExternalInput")
    aW2 = nc.dram_tensor("aW2", [NLOC, 2, 128, D], F32, kind="ExternalInput")
    vb2 = nc.dram_tensor("vb2", [NLOC, D], F32, kind="ExternalInput")
    ab2 = nc.dram_tensor("ab2", [NLOC, D], F32, kind="ExternalInput")
    # row-base offsets (into the half-split qvall) for each (set, slot)
    gbase = nc.dram_tensor("gbase", [1, SLOC * NSLOT], I32,
                           kind="ExternalInput")
    pw = nc.dram_tensor("pw", [128, SLOC, 7, 4], BF16, kind="ExternalInput")
    sw = nc.dram_tensor("sw", [128, SLOC, 3, 4], BF16, kind="ExternalInput")
    ident = nc.dram_tensor("ident", [128, 128], F32, kind="ExternalInput")

    chi = nc.dram_tensor("chi", [1, NINST * B], F32, kind="ExternalOutput")
    if DEBUG:
        dbg_x = nc.dram_tensor("dbg_x", [128, NSLOT, 2, B], BF16,
                               kind="ExternalOutput")
        dbg_pm = nc.dram_tensor("dbg_pm", [128, NPAIR, 2, B], BF16,
                                kind="ExternalOutput")

    with tile.TileContext(nc, num_cores=NCORE) as tc:
        with tc.tile_pool(name="const", bufs=1) as cpool, \
             tc.tile_pool(name="dram", bufs=1, space="DRAM") as dram:
            ident_s = cpool.tile([128, 128], F32)
            nc.sync.dma_start(out=ident_s[:], in_=ident[:])
            ones_row = cpool.tile([1, 128], F32)
            nc.vector.memset(ones_row[:], 1.0)
            eps_t = cpool.tile([B, 1], F32)
            nc.vector.memset(eps_t[:], 1e-5)
            lnV_s = cpool.tile([B, 4], F32)
            nc.sync.dma_start(out=lnV_s[:], in_=lnV[:])
            lnA_s = cpool.tile([B, 4], F32)
            nc.sync.dma_start(out=lnA_s[:], in_=lnA[:])
            pw_s = cpool.tile([128, SLOC, 7, 4], BF16)
            nc.sync.dma_start(out=pw_s[:], in_=pw[:])
            sw_s = cpool.tile([128, SLOC, 3, 4], BF16)
            nc.sync.dma_start(out=sw_s[:], in_=sw[:])
            gbase_s = cpool.tile([1, SLOC * NSLOT], I32)
            nc.sync.dma_start(out=gbase_s[:], in_=gbase[:])

            qvhalf = [dram.tile([NH, D, 2, B], BF16, name="qvh0", tag="qh0"),
                      dram.tile([NH, D, 2, B], BF16, name="qvh1", tag="qh1")]
            # [half, core, node_in_half, d, {q,v}, b]
            qvall = dram.tile([2, NCORE, NH, D, 2, B], BF16)

            # ================= Phase 1: per-node MLPs =================
            with tc.tile_pool(name="p1", bufs=3) as p1, \
                 tc.tile_pool(name="p1w", bufs=3) as p1w, \
                 tc.tile_pool(name="ps_h", bufs=3, space="PSUM") as ps_h, \
                 tc.tile_pool(name="ps_t", bufs=2, space="PSUM") as ps_t, \
                 tc.tile_pool(name="ps_o", bufs=2, space="PSUM") as ps_o:

                def mlp(i, xt_tiles, w1_dram, b1_dram, ln_s, w2_dram, b2_dram,
                        nchunk):
                    # mm1: psum_h[b, o]
                    w1 = p1w.tile([128, nchunk, H], F32, tag="w1")
                    nc.sync.dma_start(
                        out=w1[:],
                        in_=w1_dram[i].rearrange("(c p) o -> p c o", p=128))
                    b1 = p1w.tile([1, H], F32, tag="b1")
                    nc.sync.dma_start(out=b1[:], in_=b1_dram[i][None, :])
                    h = ps_h.tile([B, H], F32, tag="h")
                    for c in range(nchunk):
                        nc.tensor.matmul(h[:], xt_tiles[c], w1[:, c, :],
                                         start=(c == 0), stop=False)
                    nc.tensor.matmul(h[:], ones_row[:], b1[:],
                                     start=False, stop=True)
                    # LN stats via bn_stats/bn_aggr -> [mu, var]
                    bn6 = p1.tile([B, 6], F32, tag="bn6")
                    nc.vector.bn_stats(bn6[:], h[:])
                    bn2 = p1.tile([B, 2], F32, tag="bn2")
                    nc.vector.bn_aggr(bn2[:], bn6[:])
                    # rs = 1/sqrt(var+eps) = exp(-0.5*ln(var+eps))
                    lv = p1.tile([B, 1], F32, tag="lv")
                    nc.scalar.activation(lv[:], bn2[:, 1:2],
                                         mybir.ActivationFunctionType.Ln,
                                         bias=eps_t[:])
                    rs = p1.tile([B, 1], F32, tag="rs")
                    nc.scalar.activation(rs[:], lv[:],
                                         mybir.ActivationFunctionType.Exp,
                                         scale=-0.5)
                    # apply (h-mu)*rs
                    u = p1.tile([B, H], F32, tag="u")
                    nc.vector.tensor_scalar(u[:], h[:], bn2[:, 0:1], rs[:],
                                            mybir.AluOpType.subtract,
                                            mybir.AluOpType.mult)
                    # transpose u -> uT, relu(g*uT+be) -> hT
                    ut = ps_t.tile([128, 2, 128], F32, tag="ut")
                    for c in range(2):
                        nc.tensor.transpose(ut[:, c, :],
                                            u[:, c * 128:(c + 1) * 128],
                                            ident_s[:])
                    hT = p1.tile([128, 2, 128], F32, tag="hT")
                    for c in range(2):
                        nc.scalar.activation(hT[:, c, :], ut[:, c, :],
                                             mybir.ActivationFunctionType.Relu,
                                             bias=ln_s[:, 2 + c:3 + c],
                                             scale=ln_s[:, c:c + 1])
                    # mm2: out[d, b]
                    w2 = p1w.tile([128, 2, D], F32, tag="w2")
                    nc.sync.dma_start(
                        out=w2[:],
                        in_=w2_dram[i].rearrange("c p d -> p c d"))
                    b2 = p1w.tile([1, D], F32, tag="b2")
                    nc.sync.dma_start(out=b2[:], in_=b2_dram[i][None, :])
                    o = ps_o.tile([D, B], F32, tag="o")
                    for c in range(2):
                        nc.tensor.matmul(o[:], w2[:, c, :], hT[:, c, :],
                                         start=(c == 0), stop=False)
                    nc.tensor.matmul(o[:], b2[:], ones_row[:],
                                     start=False, stop=True)
                    return o

                for i in range(NLOC):
                    xv = p1.tile([128, 2, B], F32, tag="xv")
                    nc.sync.dma_start(
                        out=xv[:],
                        in_=obsT[i].rearrange("(c p) b -> p c b", p=128))
                    xa = p1.tile([128, 2, B], F32, tag="xa")
                    nc.sync.dma_start(
                        out=xa[:],
                        in_=actT[i].rearrange("(c p) b -> p c b", p=128))
                    ov = mlp(i, [xv[:, 0, :], xv[:, 1, :]],
                             vW1, vb1, lnV_s, vW2, vb2, 2)
                    oa = mlp(i, [xv[:, 0, :], xv[:, 1, :],
                                 xa[:, 0, :], xa[:, 1, :]],
                             aW1, ab1, lnA_s, aW2, ab2, 4)
                    vf = p1.tile([D, B], F32, tag="vf")
                    nc.scalar.copy(vf[:], ov[:])
                    qb = p1.tile([D, B], BF16, tag="qb")
                    nc.vector.tensor_tensor(qb[:], oa[:], vf[:],
                                            mybir.AluOpType.add)
                    vb = p1.tile([D, B], BF16, tag="vb")
                    nc.vector.tensor_scalar(vb[:], vf[:], 1.0, None,
                                            mybir.AluOpType.mult)
                    half, ih = divmod(i, NH)
                    nc.sync.dma_start(out=qvhalf[half][ih, :, 0, :], in_=qb[:])
                    nc.sync.dma_start(out=qvhalf[half][ih, :, 1, :], in_=vb[:])
                    # ======== Phase 2: split AllGather (overlapped) ========
                    if i == NH - 1 or i == NLOC - 1:
                        half = 0 if i == NH - 1 else 1
                        nc.gpsimd.collective_compute(
                            "AllGather", mybir.AluOpType.bypass,
                            replica_groups=[list(range(NCORE))],
                            ins=[qvhalf[half].opt()],
                            outs=[qvall[half].opt()],
                        )

            # flat rows: row = ((half*8 + core)*NH + n)*128 + d, each [2*B]
            qv_flat = qvall.rearrange("h c n p t b -> (h c n p) (t b)")

            # ================= Phase 3: Choquet =================
            SG = 2  # sets per group (shared X/PM tiles)
            with tc.tile_pool(name="p3", bufs=3) as p3, \
                 tc.tile_pool(name="p3pm", bufs=2) as p3pm, \
                 tc.tile_pool(name="ps_p", bufs=4, space="PSUM") as ps_p, \
                 tc.tile_pool(name="ps_r", bufs=1, space="PSUM") as ps_r, \
                 tc.tile_pool(name="p3c", bufs=1) as p3c:
                chirow = ps_r.tile([1, NINST * B], F32)
                chi4 = [p3c.tile([4, SLOC, 4, B], F32,
                                 name="chi4q", tag="chi4q"),
                        p3c.tile([4, SLOC, 4, B], F32,
                                 name="chi4v", tag="chi4v")]
                for g in range(SLOC // SG):
                    X = p3.tile([128, SG, NSLOT, 2, B], BF16, tag="X")
                    # contiguous 64KB block DMAs with register offsets,
                    # alternating the two HWDGE queues (sync / scalar)
                    eng_t, eng = ((mybir.EngineType.SP, nc.sync)
                                  if g % 2 == 0 else
                                  (mybir.EngineType.Activation, nc.scalar))
                    j0 = g * SG * NSLOT
                    _, vals = nc.values_load_multi_w_load_instructions(
                        gbase_s[0:1, j0:j0 + SG * NSLOT],
                        engines=[eng_t],
                        skip_runtime_bounds_check=True)
                    for sl in range(SG):
                        for k in range(NSLOT):
                            eng.dma_start(
                                out=X[:, sl, k, :, :],
                                in_=qv_flat[
                                    bass.ds(vals[sl * NSLOT + k], 128), :])
                    PM = p3pm.tile([128, SG, NPAIR, 2, B], BF16, tag="PM")
                    off = 0
                    for dd in range(1, K):
                        n = K - dd
                        nc.vector.tensor_tensor(
                            PM[:, :, off:off + n, :, :],
                            X[:, :, 1:1 + n, :, :],
                            X[:, :, 1 + dd:1 + dd + n, :, :],
                            mybir.AluOpType.min)
                        off += n
                    if DEBUG and g == 0:
                        nc.sync.dma_start(out=dbg_x[:], in_=X[:, 0])
                        nc.sync.dma_start(out=dbg_pm[:], in_=PM[:, 0])
                    for sl in range(SG):
                        s = g * SG + sl
                        for t in range(2):
                            P = ps_p.tile([4, 4 * B], F32, tag="P")
                            for j in range(7):
                                nc.tensor.matmul(
                                    P[:], pw_s[:, s, j, :],
                                    PM[:, sl, 4 * j:4 * j + 4, t, :],
                                    start=(j == 0), stop=False)
                            for j, s0 in enumerate((0, 4, 5)):
                                nc.tensor.matmul(
                                    P[:], sw_s[:, s, j, :],
                                    X[:, sl, s0:s0 + 4, t, :],
                                    start=False, stop=(j == 2))
                            dst = chi4[t][:, s, :, :].rearrange(
                                "p a b -> p (a b)")
                            if t == 0:
                                nc.scalar.copy(dst, P[:])
                            else:
                                nc.vector.tensor_copy(dst, P[:])
                # fold: extract diagonal blocks P[c, c, b]; batch 4 sets/MM
                for t in range(2):
                    for g4 in range(SLOC // 4):
                        for c in range(4):
                            nc.tensor.matmul(
                                chirow[:, (t * SLOC + g4 * 4) * B:
                                       (t * SLOC + g4 * 4 + 4) * B],
                                ident_s[:4, c:c + 1],
                                chi4[t][:, 4 * g4:4 * g4 + 4, c, :],
                                start=(c == 0), stop=(c == 3))
                chirow_s = p3c.tile([1, NINST * B], F32)
                nc.scalar.copy(chirow_s[:], chirow[:])
                nc.sync.dma_start(out=chi[:], in_=chirow_s[:])

    nc.compile()
    return nc


def _prepare_inputs(observation, action, local_edges, V_W1, V_b1, V_g1,
                    V_beta1, V_W2, V_b2, A_W1, A_b1, A_g1, A_beta1, A_W2,
                    A_b2, chi_m1, chi_m2):
    centers = np.asarray(local_edges[:, 0, 0]).astype(np.int64)
    neigh = np.asarray(local_edges[:, 0, 1:]).astype(np.int64)
    m1s = chi_m1.sum(1) / (HEADS * D)              # [S, K]
    tri = np.triu(np.ones((K, K), np.float32), k=1)
    m2s = (chi_m2.sum(1) * tri) / (HEADS * D)      # [S, K, K]

    in_maps = []
    for c in range(NCORE):
        nodes = slice(c * NLOC, (c + 1) * NLOC)
        obsn = observation[:, nodes, :]            # [B, 8, H]
        actn = action[:, nodes, :]
        m = {}
        m["obsT"] = np.ascontiguousarray(obsn.transpose(1, 2, 0))
        m["actT"] = np.ascontiguousarray(actn.transpose(1, 2, 0))
        m["vW1"] = np.ascontiguousarray(V_W1[nodes])
        m["vb1"] = np.ascontiguousarray(V_b1[nodes])
        m["aW1"] = np.ascontiguousarray(A_W1[nodes])
        m["ab1"] = np.ascontiguousarray(A_b1[nodes])
        lnv = np.zeros((B, 4), np.float32)
        lnv[:, 0] = V_g1[:128]; lnv[:, 1] = V_g1[128:]
        lnv[:, 2] = V_beta1[:128]; lnv[:, 3] = V_beta1[128:]
        m["lnV"] = lnv
        lna = np.zeros((B, 4), np.float32)
        lna[:, 0] = A_g1[:128]; lna[:, 1] = A_g1[128:]
        lna[:, 2] = A_beta1[:128]; lna[:, 3] = A_beta1[128:]
        m["lnA"] = lna
        m["vW2"] = np.ascontiguousarray(
            V_W2[nodes].reshape(NLOC, 2, 128, D))
        m["aW2"] = np.ascontiguousarray(
            A_W2[nodes].reshape(NLOC, 2, 128, D))
        m["vb2"] = np.ascontiguousarray(V_b2[nodes])
        m["ab2"] = np.ascontiguousarray(A_b2[nodes])

        gb = np.zeros((1, SLOC * NSLOT), np.int32)
        pwn = np.zeros((SLOC, 7, 4), np.float32)
        swn = np.zeros((SLOC, 3, 4), np.float32)
        for sl in range(SLOC):
            s = c * SLOC + sl
            slots = [int(centers[s])] + [int(x) for x in neigh[s]]
            for k in range(NSLOT):
                node = slots[k]
                half, nh = divmod(node % NLOC, NH)
                row = ((half * NCORE + node // NLOC) * NH + nh) * D
                gb[0, sl * NSLOT + k] = row
            for p, (a, b_) in enumerate(PAIRS):
                pwn[sl, p // 4, p % 4] = m2s[s, a - 1, b_ - 1]
            # device slot groups: j=0 slots 0-3, j=1 slots 4-7, j=2 slots 5-8
            swn[sl, 0, 0] = 1.0 / D                # center
            for k in range(1, 8):
                swn[sl, k // 4, k % 4] = m1s[s, k - 1]
            swn[sl, 2, 3] = m1s[s, 7]              # slot 8
        m["gbase"] = gb
        m["pw"] = np.broadcast_to(
            pwn.astype(ml_dtypes.bfloat16)[None], (128, SLOC, 7, 4)).copy()
        m["sw"] = np.broadcast_to(
            swn.astype(ml_dtypes.bfloat16)[None], (128, SLOC, 3, 4)).copy()
        m["ident"] = np.eye(128, dtype=np.float32)
        in_maps.append(m)
    return in_maps


def kernel(**inputs):
    global _compiled
    if _compiled is None:
        _compiled = _build()
    nc = _compiled
    inputs = {k: np.asarray(v) for k, v in inputs.items()}
    in_maps = _prepare_inputs(**inputs)
    res = run_bass_kernel_spmd(nc, in_maps, list(range(NCORE)))
    global _last_results
    _last_results = res
    chi_q = np.zeros((B, N), np.float32)
    chi_v = np.zeros((B, N), np.float32)
    for c in range(NCORE):
        out = res.results[c]["chi"].reshape(NINST, B)
        for sl in range(SLOC):
            chi_q[:, c * SLOC + sl] = out[sl]
            chi_v[:, c * SLOC + sl] = out[SLOC + sl]
    return chi_q, chi_v


# revision 30
# speedup vs baseline: 1.6007x; 1.0732x over previous
"""Trainium2 Bass kernel for nn_Critic (gnn_message_passing).

Strategy (8 NeuronCores, one SPMD NEFF):
  - Node-shard the per-node MLPs (8 nodes/core) so the big per-node weights
    (67MB total) are read once across the chip instead of replicated.
  - mm1 in [b, o] layout, LN stats via DVE bn_stats/bn_aggr, rsqrt as
    exp(-0.5*ln(var+eps)) (keeps every ACT function in one LUT table-set),
    PE transpose, fused scale/bias/relu on ACT, mm2 in [d, b] layout.
  - Q=A+V and V are packed [node, d, {q,v}, b] in bf16 and AllGathered in
    two halves so the first collective overlaps the second half of the MLP
    compute.
  - Choquet phase is set-sharded (8 sets/core; Q and V ride together):
    neighbor blocks fetched as contiguous 64KB register-offset HWDGE DMAs
    (offsets from an int32 input -> batched reg loads, alternating the
    sync/scalar DMA queues), pair terms via delta-batched DVE mins over
    two-set groups, and all d-reductions as PSUM-accumulated weighted
    matmuls on the Tensor engine (diagonal-block extraction).
All per-core-varying structure (gather offsets, Mobius coefficients) enters
as input data so the single SPMD program stays uniform across cores.
"""

import os

import numpy as np
import ml_dtypes

import concourse.bass as bass
import concourse.bacc as bacc
import concourse.mybir as mybir
from concourse import tile
from concourse.bass_utils import run_bass_kernel_spmd

DEBUG = bool(os.environ.get("KERNEL_DEBUG"))

B, N, H, D, K, HEADS = 128, 64, 256, 128, 8, 3
NCORE = 8
NLOC = N // NCORE      # nodes per core
SLOC = N // NCORE      # sets per core
NINST = 2 * SLOC       # choquet instances per core (Q sets then V sets)
NSLOT = K + 1          # center + 8 neighbors
NPAIR = (K * (K - 1)) // 2  # 28
NH = NLOC // 2         # nodes per collective half
F32 = mybir.dt.float32
BF16 = mybir.dt.bfloat16
I32 = mybir.dt.int32

# pairs in delta-major order over neighbor slots 1..8
PAIRS = [(a, a + d) for d in range(1, K) for a in range(1, K - d + 1)]

_compiled = None


def _build():
    nc = bacc.Bacc("TRN2", target_bir_lowering=False, debug=False,
                   num_devices=NCORE)

    # ---- per-core inputs ----
    obsT = nc.dram_tensor("obsT", [NLOC, H, B], F32, kind="ExternalInput")
    actT = nc.dram_tensor("actT", [NLOC, H, B], F32, kind="ExternalInput")
    # packed weights: wpV[i, p, c, :] = [W1V_chunk(256) | W2V_chunk(128) |
    # W2A_chunk(128)]; wpA[i, p, c, :] = W1A chunk (256)
    wpV = nc.dram_tensor("wpV", [NLOC, 128, 2, 512], F32,
                         kind="ExternalInput")
    wpA = nc.dram_tensor("wpA", [NLOC, 128, 4, 256], F32,
                         kind="ExternalInput")
    # packed biases: [b1V(256) | b2V(128) | b1A(256) | b2A(128)]
    bp = nc.dram_tensor("bp", [NLOC, 768], F32, kind="ExternalInput")
    lnV = nc.dram_tensor("lnV", [B, 4], F32, kind="ExternalInput")
    lnA = nc.dram_tensor("lnA", [B, 4], F32, kind="ExternalInput")
    # row-base offsets (into the half-split qvall) for each (set, slot)
    gbase = nc.dram_tensor("gbase", [1, SLOC * NSLOT], I32,
                           kind="ExternalInput")
    pw = nc.dram_tensor("pw", [128, SLOC, 7, 4], BF16, kind="ExternalInput")
    sw = nc.dram_tensor("sw", [128, SLOC, 3, 4], BF16, kind="ExternalInput")
    ident = nc.dram_tensor("ident", [128, 128], F32, kind="ExternalInput")

    chi = nc.dram_tensor("chi", [1, NINST * B], F32, kind="ExternalOutput")
    if DEBUG:
        dbg_x = nc.dram_tensor("dbg_x", [128, NSLOT, 2, B], BF16,
                               kind="ExternalOutput")
        dbg_pm = nc.dram_tensor("dbg_pm", [128, NPAIR, 2, B], BF16,
                                kind="ExternalOutput")

    with tile.TileContext(nc, num_cores=NCORE) as tc:
        with tc.tile_pool(name="const", bufs=1) as cpool, \
             tc.tile_pool(name="dram", bufs=1, space="DRAM") as dram:
            ident_s = cpool.tile([128, 128], F32)
            nc.sync.dma_start(out=ident_s[:], in_=ident[:])
            ones_row = cpool.tile([1, 128], F32)
            nc.vector.memset(ones_row[:], 1.0)
            eps_t = cpool.tile([B, 1], F32)
            nc.vector.memset(eps_t[:], 1e-5)
            lnV_s = cpool.tile([B, 4], F32)
            nc.sync.dma_start(out=lnV_s[:], in_=lnV[:])
            lnA_s = cpool.tile([B, 4], F32)
            nc.sync.dma_start(out=lnA_s[:], in_=lnA[:])
            pw_s = cpool.tile([128, SLOC, 7, 4], BF16)
            nc.sync.dma_start(out=pw_s[:], in_=pw[:])
            sw_s = cpool.tile([128, SLOC, 3, 4], BF16)
            nc.sync.dma_start(out=sw_s[:], in_=sw[:])
            gbase_s = cpool.tile([1, SLOC * NSLOT], I32)
            nc.sync.dma_start(out=gbase_s[:], in_=gbase[:])

            qvloc = dram.tile([NLOC, D, 2, B], BF16)
            # [core, node, d, {q,v}, b]
            qvall = dram.tile([NCORE, NLOC, D, 2, B], BF16,
                              addr_space="Shared")

            # ================= Phase 1: per-node MLPs =================
            with tc.tile_pool(name="p1", bufs=3) as p1, \
                 tc.tile_pool(name="p1w", bufs=3) as p1w, \
                 tc.tile_pool(name="ps_h", bufs=3, space="PSUM") as ps_h, \
                 tc.tile_pool(name="ps_t", bufs=2, space="PSUM") as ps_t, \
                 tc.tile_pool(name="ps_o", bufs=2, space="PSUM") as ps_o:

                def mlp(xt_tiles, w1_aps, b1_ap, ln_s, w2_aps, b2_ap):
                    # mm1: psum_h[b, o]
                    h = ps_h.tile([B, H], F32, tag="h")
                    for c, (xt, w1c) in enumerate(zip(xt_tiles, w1_aps)):
                        nc.tensor.matmul(h[:], xt, w1c,
                                         start=(c == 0), stop=False)
                    nc.tensor.matmul(h[:], ones_row[:], b1_ap,
                                     start=False, stop=True)
                    # LN stats via bn_stats/bn_aggr -> [mu, var]
                    bn6 = p1.tile([B, 6], F32, tag="bn6")
                    nc.vector.bn_stats(bn6[:], h[:])
                    bn2 = p1.tile([B, 2], F32, tag="bn2")
                    nc.vector.bn_aggr(bn2[:], bn6[:])
                    # rs = 1/sqrt(var+eps) = exp(-0.5*ln(var+eps))
                    lv = p1.tile([B, 1], F32, tag="lv")
                    nc.scalar.activation(lv[:], bn2[:, 1:2],
                                         mybir.ActivationFunctionType.Ln,
                                         bias=eps_t[:])
                    rs = p1.tile([B, 1], F32, tag="rs")
                    nc.scalar.activation(rs[:], lv[:],
                                         mybir.ActivationFunctionType.Exp,
                                         scale=-0.5)
                    # apply (h-mu)*rs
                    u = p1.tile([B, H], F32, tag="u")
                    nc.vector.tensor_scalar(u[:], h[:], bn2[:, 0:1], rs[:],
                                            mybir.AluOpType.subtract,
                                            mybir.AluOpType.mult)
                    # transpose u -> uT; hT = relu(g*uT + be) on DVE
                    ut = ps_t.tile([128, 2, 128], F32, tag="ut")
                    for c in range(2):
                        nc.tensor.transpose(ut[:, c, :],
                                            u[:, c * 128:(c + 1) * 128],
                                            ident_s[:])
                    hT = p1.tile([128, 2, 128], F32, tag="hT")
                    for c in range(2):
                        nc.vector.tensor_scalar(
                            hT[:, c, :], ut[:, c, :],
                            ln_s[:, c:c + 1], ln_s[:, 2 + c:3 + c],
                            mybir.AluOpType.mult, mybir.AluOpType.add)
                        nc.vector.tensor_scalar(
                            hT[:, c, :], hT[:, c, :], 0.0, None,
                            mybir.AluOpType.max)
                    # mm2: out[d, b]
                    o = ps_o.tile([D, B], F32, tag="o")
                    for c in range(2):
                        nc.tensor.matmul(o[:], w2_aps[c], hT[:, c, :],
                                         start=(c == 0), stop=False)
                    nc.tensor.matmul(o[:], b2_ap, ones_row[:],
                                     start=False, stop=True)
                    return o

                for i in range(NLOC):
                    xv = p1.tile([128, 2, B], F32, tag="xv")
                    nc.sync.dma_start(
                        out=xv[:],
                        in_=obsT[i].rearrange("(c p) b -> p c b", p=128))
                    xa = p1.tile([128, 2, B], F32, tag="xa")
                    nc.scalar.dma_start(
                        out=xa[:],
                        in_=actT[i].rearrange("(c p) b -> p c b", p=128))
                    wv = p1w.tile([128, 2, 512], F32, tag="wv")
                    nc.sync.dma_start(out=wv[:], in_=wpV[i])
                    wa = p1w.tile([128, 4, 256], F32, tag="wa")
                    nc.scalar.dma_start(out=wa[:], in_=wpA[i])
                    bt = p1w.tile([1, 768], F32, tag="bt")
                    nc.sync.dma_start(out=bt[:], in_=bp[i][None, :])
                    ov = mlp([xv[:, 0, :], xv[:, 1, :]],
                             [wv[:, 0, 0:256], wv[:, 1, 0:256]],
                             bt[:, 0:256], lnV_s,
                             [wv[:, 0, 256:384], wv[:, 1, 256:384]],
                             bt[:, 256:384])
                    oa = mlp([xv[:, 0, :], xv[:, 1, :],
                              xa[:, 0, :], xa[:, 1, :]],
                             [wa[:, 0, :], wa[:, 1, :],
                              wa[:, 2, :], wa[:, 3, :]],
                             bt[:, 384:640], lnA_s,
                             [wv[:, 0, 384:512], wv[:, 1, 384:512]],
                             bt[:, 640:768])
                    vf = p1.tile([D, B], F32, tag="vf")
                    nc.scalar.copy(vf[:], ov[:])
                    qb = p1.tile([D, B], BF16, tag="qb")
                    nc.vector.tensor_tensor(qb[:], oa[:], vf[:],
                                            mybir.AluOpType.add)
                    vb = p1.tile([D, B], BF16, tag="vb")
                    nc.vector.tensor_scalar(vb[:], vf[:], 1.0, None,
                                            mybir.AluOpType.mult)
                    nc.sync.dma_start(out=qvloc[i, :, 0, :], in_=qb[:])
                    nc.scalar.dma_start(out=qvloc[i, :, 1, :], in_=vb[:])

            # ================= Phase 2: AllGather =================
            nc.gpsimd.collective_compute(
                "AllGather", mybir.AluOpType.bypass,
                replica_groups=[list(range(NCORE))],
                ins=[qvloc.opt()], outs=[qvall.opt()],
            )
            # flat rows: row = node*128 + d, each [2*B]
            qv_flat = qvall.rearrange("c n p t b -> (c n p) (t b)")

            # ================= Phase 3: Choquet =================
            SG = 2  # sets per group (shared X/PM tiles)
            with tc.tile_pool(name="p3", bufs=3) as p3, \
                 tc.tile_pool(name="p3pm", bufs=2) as p3pm, \
                 tc.tile_pool(name="ps_p", bufs=4, space="PSUM") as ps_p, \
                 tc.tile_pool(name="ps_r", bufs=1, space="PSUM") as ps_r, \
                 tc.tile_pool(name="p3c", bufs=1) as p3c:
                chirow = ps_r.tile([1, NINST * B], F32)
                chi4 = [p3c.tile([4, SLOC, 4, B], F32,
                                 name="chi4q", tag="chi4q"),
                        p3c.tile([4, SLOC, 4, B], F32,
                                 name="chi4v", tag="chi4v")]
                for g in range(SLOC // SG):
                    X = p3.tile([128, SG, NSLOT, 2, B], BF16, tag="X")
                    # contiguous 64KB block DMAs with register offsets,
                    # alternating the two HWDGE queues (sync / scalar)
                    eng_t, eng = ((mybir.EngineType.SP, nc.sync)
                                  if g % 2 == 0 else
                                  (mybir.EngineType.Activation, nc.scalar))
                    j0 = g * SG * NSLOT
                    _, vals = nc.values_load_multi_w_load_instructions(
                        gbase_s[0:1, j0:j0 + SG * NSLOT],
                        engines=[eng_t],
                        skip_runtime_bounds_check=True)
                    for sl in range(SG):
                        for k in range(NSLOT):
                            eng.dma_start(
                                out=X[:, sl, k, :, :],
                                in_=qv_flat[
                                    bass.ds(vals[sl * NSLOT + k], 128), :])
                    PM = p3pm.tile([128, SG, NPAIR, 2, B], BF16, tag="PM")
                    off = 0
                    for dd in range(1, K):
                        n = K - dd
                        nc.vector.tensor_tensor(
                            PM[:, :, off:off + n, :, :],
                            X[:, :, 1:1 + n, :, :],
                            X[:, :, 1 + dd:1 + dd + n, :, :],
                            mybir.AluOpType.min)
                        off += n
                    if DEBUG and g == 0:
                        nc.sync.dma_start(out=dbg_x[:], in_=X[:, 0])
                        nc.sync.dma_start(out=dbg_pm[:], in_=PM[:, 0])
                    for sl in range(SG):
                        s = g * SG + sl
                        for t in range(2):
                            P = ps_p.tile([4, 4 * B], F32, tag="P")
                            for j in range(7):
                                nc.tensor.matmul(
                                    P[:], pw_s[:, s, j, :],
                                    PM[:, sl, 4 * j:4 * j + 4, t, :],
                                    start=(j == 0), stop=False)
                            for j, s0 in enumerate((0, 4, 5)):
                                nc.tensor.matmul(
                                    P[:], sw_s[:, s, j, :],
                                    X[:, sl, s0:s0 + 4, t, :],
                                    start=False, stop=(j == 2))
                            dst = chi4[t][:, s, :, :].rearrange(
                                "p a b -> p (a b)")
                            if t == 0:
                                nc.scalar.copy(dst, P[:])
                            else:
                                nc.vector.tensor_copy(dst, P[:])
                # fold: extract diagonal blocks P[c, c, b]; batch 4 sets/MM
                for t in range(2):
                    for g4 in range(SLOC // 4):
                        for c in range(4):
                            nc.tensor.matmul(
                                chirow[:, (t * SLOC + g4 * 4) * B:
                                       (t * SLOC + g4 * 4 + 4) * B],
                                ident_s[:4, c:c + 1],
                                chi4[t][:, 4 * g4:4 * g4 + 4, c, :],
                                start=(c == 0), stop=(c == 3))
                chirow_s = p3c.tile([1, NINST * B], F32)
                nc.scalar.copy(chirow_s[:], chirow[:])
                nc.sync.dma_start(out=chi[:], in_=chirow_s[:])

    nc.compile()
    return nc


def _prepare_inputs(observation, action, local_edges, V_W1, V_b1, V_g1,
                    V_beta1, V_W2, V_b2, A_W1, A_b1, A_g1, A_beta1, A_W2,
                    A_b2, chi_m1, chi_m2):
    centers = np.asarray(local_edges[:, 0, 0]).astype(np.int64)
    neigh = np.asarray(local_edges[:, 0, 1:]).astype(np.int64)
    m1s = chi_m1.sum(1) / (HEADS * D)              # [S, K]
    tri = np.triu(np.ones((K, K), np.float32), k=1)
    m2s = (chi_m2.sum(1) * tri) / (HEADS * D)      # [S, K, K]

    in_maps = []
    for c in range(NCORE):
        nodes = slice(c * NLOC, (c + 1) * NLOC)
        obsn = observation[:, nodes, :]            # [B, 8, H]
        actn = action[:, nodes, :]
        m = {}
        m["obsT"] = np.ascontiguousarray(obsn.transpose(1, 2, 0))
        m["actT"] = np.ascontiguousarray(actn.transpose(1, 2, 0))
        # packed weights: wpV[i, p, c, :] = [W1V[c*128+p, :256] |
        #                                    W2V[c*128+p, :] | W2A[c*128+p, :]]
        w1v = V_W1[nodes].reshape(NLOC, 2, 128, H).transpose(0, 2, 1, 3)
        w2v = V_W2[nodes].reshape(NLOC, 2, 128, D).transpose(0, 2, 1, 3)
        w2a = A_W2[nodes].reshape(NLOC, 2, 128, D).transpose(0, 2, 1, 3)
        m["wpV"] = np.ascontiguousarray(
            np.concatenate([w1v, w2v, w2a], axis=3))
        m["wpA"] = np.ascontiguousarray(
            A_W1[nodes].reshape(NLOC, 4, 128, H).transpose(0, 2, 1, 3))
        m["bp"] = np.ascontiguousarray(np.concatenate(
            [V_b1[nodes], V_b2[nodes], A_b1[nodes], A_b2[nodes]], axis=1))
        lnv = np.zeros((B, 4), np.float32)
        lnv[:, 0] = V_g1[:128]; lnv[:, 1] = V_g1[128:]
        lnv[:, 2] = V_beta1[:128]; lnv[:, 3] = V_beta1[128:]
        m["lnV"] = lnv
        lna = np.zeros((B, 4), np.float32)
        lna[:, 0] = A_g1[:128]; lna[:, 1] = A_g1[128:]
        lna[:, 2] = A_beta1[:128]; lna[:, 3] = A_beta1[128:]
        m["lnA"] = lna

        gb = np.zeros((1, SLOC * NSLOT), np.int32)
        pwn = np.zeros((SLOC, 7, 4), np.float32)
        swn = np.zeros((SLOC, 3, 4), np.float32)
        for sl in range(SLOC):
            s = c * SLOC + sl
            slots = [int(centers[s])] + [int(x) for x in neigh[s]]
            for k in range(NSLOT):
                gb[0, sl * NSLOT + k] = slots[k] * D
            for p, (a, b_) in enumerate(PAIRS):
                pwn[sl, p // 4, p % 4] = m2s[s, a - 1, b_ - 1]
            # device slot groups: j=0 slots 0-3, j=1 slots 4-7, j=2 slots 5-8
            swn[sl, 0, 0] = 1.0 / D                # center
            for k in range(1, 8):
                swn[sl, k // 4, k % 4] = m1s[s, k - 1]
            swn[sl, 2, 3] = m1s[s, 7]              # slot 8
        m["gbase"] = gb
        m["pw"] = np.broadcast_to(
            pwn.astype(ml_dtypes.bfloat16)[None], (128, SLOC, 7, 4)).copy()
        m["sw"] = np.broadcast_to(
            swn.astype(ml_dtypes.bfloat16)[None], (128, SLOC, 3, 4)).copy()
        m["ident"] = np.eye(128, dtype=np.float32)
        in_maps.append(m)
    return in_maps


def kernel(**inputs):
    global _compiled
    if _compiled is None:
        _compiled = _build()
    nc = _compiled
    inputs = {k: np.asarray(v) for k, v in inputs.items()}
    in_maps = _prepare_inputs(**inputs)
    res = run_bass_kernel_spmd(nc, in_maps, list(range(NCORE)))
    global _last_results
    _last_results = res
    chi_q = np.zeros((B, N), np.float32)
    chi_v = np.zeros((B, N), np.float32)
    for c in range(NCORE):
        out = res.results[c]["chi"].reshape(NINST, B)
        for sl in range(SLOC):
            chi_q[:, c * SLOC + sl] = out[sl]
            chi_v[:, c * SLOC + sl] = out[SLOC + sl]
    return chi_q, chi_v
